# revision 5
# baseline (speedup 1.0000x reference)
"""Trainium2 Bass kernel for nn_EpsiLayer: per-channel causal full-length
time convolution  out[b,t,j] = P[b,t,j] + sum_{k<=t} g[k,j] * P[b,t-k,j].

Identity fold: with g'[0] = g[0] + 1, out = causal_conv(g', P) exactly.
Per channel j the conv is a lower-triangular Toeplitz (T x T) matmul,
blocked into C=128 chunks: y_i += W_d @ x_{i-d} with Hankel tiles
W_d[p, a] = gpad[d*128 + a + p], gpad = 127 zeros ++ g' (bf16).
Sharding: channel-parallel, 32 channels/core on 8 cores, no comms.

The kernel sits at a measured double roofline (~113-116 us wall,
+-5-15 us shared-HBM environment variance):
  - PE: per-matmul ~ 30 ns fixed + 0.41 ns/col (microbenchmarked;
    INDEPENDENT of weight reuse / FWL / explicit ldweights - the
    embedded weight load is already hidden by the PE reorder window).
    Conv = 32ch x 32 MMs, avg N=132 -> ~86 us; tail-tile generation
    (below) adds ~21 us -> PE busy ~107 us.
  - DMA: ~30.5 MB/core weight+x+out stream at ~300-358 GB/s ~ 95-101
    us.  gend0=24 (8 of 32 tiles PE-generated from seeds at 1/4 the
    dense bytes) balances the two; raising or lowering NGEN measured
    worse in matched A/B.
Tail offsets d>=24 are generated on-PE: seeds S[p,y]=gpad[d*128+p+4y]
ship packed in the weight DMA; 4 shift-matrix matmuls (z=0..3) scatter
them into PSUM columns z::4 (z-outer over 2 chunks so LDWEIGHTS hides
under the same-Sigma stream); DVE/ACT copy PSUM->SBUF one pair ahead;
bottom 3 partitions (wraparound rows) come from a tiny HBM strip (a
circular-shift stationary cannot replace them: the wrap needs the
NEXT tile's seed column).  x is loaded as XSPLIT separate tiles so
early channels depend only on their own x slice.

Measured SLOWER or neutral this session (kept out): wide-N gen MMs
with contiguous per-z PSUM + DVE stride-interleave (+5 us), finer
weight-DMA piece splitting (+25 us), PE warmup MMs (+2 us), x-DMA
leading the weight stream (+4 us), ngen 4/6/dense-prologue variants
(+3-8 us), out-DMA on the sync ring (neutral), final-flush split
(neutral), fp8/int8 weights (fail 2e-2 error budget / no int8 matmul
in Bass).
"""

import os
import sys
import numpy as np

os.environ.setdefault("JAX_PLATFORMS", "cpu")

try:
    from concourse import bacc, tile  # noqa: F401
except ImportError:
    sys.path.insert(0, "/opt/trn_rl_repo")

import ml_dtypes

B, T, NR = 8, 4096, 256
C = 128
NB = T // C
N_CORES = 8
CPC = NR // N_CORES
COLS = CPC * NB * B
GLEN = 127 + T + 1

_cache = {}


def _build_nc(reps=1, OB=2, XSPLIT=8, oeng="scalar", wbufs=8,
              pbufs=3, obufs=4, ceng="scalar", obf16=True, warmup=30,
              gend0=25, gbufs=3, gpbufs=2, geng="vector", nz=4,
              sqeng="gpsimd", look=2, walt=False, fullgen=3, gwbufs=4,
              tailsplit=0, xtiles=0):
    from concourse import bacc, tile
    import concourse.mybir as mybir

    NZ = nz
    SEEDY = C // NZ
    GROWS = C - (NZ - 1)
    NGEN = NB - gend0 if gend0 is not None else 0
    ND = NB - NGEN
    nc = bacc.Bacc("TRN2", target_bir_lowering=False, debug=False)

    FG = fullgen if NGEN else 0
    WCOLS = ND * C + (NGEN * SEEDY if NGEN else 0)
    w_d = nc.dram_tensor("wdense", [CPC - FG, C, WCOLS], mybir.dt.bfloat16,
                         kind="ExternalInput")
    if FG:
        fseed_d = nc.dram_tensor("fseeds", [FG, C, NB * SEEDY],
                                 mybir.dt.bfloat16, kind="ExternalInput")
        fstrip_d = nc.dram_tensor("fstrips", [FG, NZ - 1, NB * C],
                                  mybir.dt.bfloat16, kind="ExternalInput")
    x_d = nc.dram_tensor("xmov", [C, COLS], mybir.dt.bfloat16,
                         kind="ExternalInput")
    if NGEN:
        strip_d = nc.dram_tensor("strips", [CPC // 2, NZ - 1, 2 * NGEN * C],
                                 mybir.dt.bfloat16, kind="ExternalInput")
        shift_d = nc.dram_tensor("shifts", [C, NZ * C], mybir.dt.bfloat16,
                                 kind="ExternalInput")
    odt = mybir.dt.bfloat16 if obf16 else mybir.dt.float32
    o_d = nc.dram_tensor("out", [C, COLS], odt, kind="ExternalOutput")

    def _copy(eng, dst, src):
        if eng == "scalar":
            nc.scalar.activation(dst, src, mybir.ActivationFunctionType.Copy)
        else:
            getattr(nc, eng).tensor_copy(dst, src)

    with tile.TileContext(nc) as tc:
        with (
            tc.tile_pool(name="xpool", bufs=1) as xpool,
            tc.tile_pool(name="wpool", bufs=wbufs) as wpool,
            tc.tile_pool(name="opool", bufs=obufs) as opool,
            tc.tile_pool(name="spool", bufs=4) as spool,
            tc.tile_pool(name="gwpool", bufs=gwbufs) as gwpool,
            tc.tile_pool(name="psum", bufs=pbufs, space="PSUM") as psum,
            tc.tile_pool(name="gpsum", bufs=gpbufs, space="PSUM") as gpsum,
            tc.tile_pool(name="wupsum", bufs=1, space="PSUM") as wupsum,
        ):
            XCH = COLS // XSPLIT
            if xtiles:
                assert XCH % (NB * B) == 0
                xts = [xpool.tile([C, XCH], mybir.dt.bfloat16,
                                  tag=f"x{k}", name=f"x{k}")
                       for k in range(XSPLIT)]
                for k in range(XSPLIT):
                    nc.scalar.dma_start(xts[k][:],
                                        x_d[:, k * XCH:(k + 1) * XCH])
            else:
                xmov = xpool.tile([C, COLS], mybir.dt.bfloat16)
                for k in range(XSPLIT):
                    nc.scalar.dma_start(xmov[:, k * XCH:(k + 1) * XCH],
                                        x_d[:, k * XCH:(k + 1) * XCH])

            if NGEN:
                shifts = xpool.tile([C, NZ * C], mybir.dt.bfloat16,
                                    tag="shifts")
                nc.sync.dma_start(shifts[:], shift_d.ap())

            if warmup:
                wu = xpool.tile([C, C], mybir.dt.bfloat16, tag="warm")
                wups = wupsum.tile([C, C], mybir.dt.float32, tag="warmp")
                nc.vector.memset(wu[:], 0)
                for _ in range(warmup):
                    nc.tensor.matmul(wups[:], wu[:], wu[:], start=True,
                                     stop=True)

            GCH = 4
            def fullgen_ch(j):
                sj = spool.tile([C, NB * SEEDY], mybir.dt.bfloat16,
                                tag="fseed")
                nc.gpsimd.dma_start(sj[:], fseed_d[j])
                fwg = gwpool.tile([C, NB * C], mybir.dt.bfloat16, tag="fwg")
                nc.gpsimd.dma_start(fwg[GROWS:C, :], fstrip_d[j])
                chunks = [(c0, min(c0 + GCH, NB)) for c0 in range(0, NB, GCH)]
                for ci in range(0, len(chunks), 2):
                    grp = chunks[ci:ci + 2]
                    pgs = [gpsum.tile([C, (c1 - c0) * C], mybir.dt.float32,
                                      tag="pg", name=f"fpg{j}_{c0}")
                           for c0, c1 in grp]
                    for z in range(NZ):
                        for (c0, c1), pg in zip(grp, pgs):
                            nc.tensor.matmul(
                                pg[:, z::NZ],
                                shifts[:, z * C:(z + 1) * C],
                                sj[:, c0 * SEEDY:c1 * SEEDY],
                                start=(z == 0),
                                stop=(z == NZ - 1),
                            )
                    for k, ((c0, c1), pg) in enumerate(zip(grp, pgs)):
                        _copy(["vector", "scalar"][(ci + k) % 2],
                              fwg[0:GROWS, c0 * C:c1 * C], pg[0:GROWS, :])
                return fwg

            pair_tiles = {}
            def gen_tiles(j):
                cpeng = ["vector", "scalar"][j % 2] if geng == "alt" else geng
                wj = wpool.tile([C, WCOLS], mybir.dt.bfloat16)
                weng = [nc.sync, nc.scalar][j % 2] if walt else nc.sync
                weng.dma_start(wj[:], w_d[j - FG])
                sj = wj
                soff = ND * C
                if j % 2 == 0:
                    wgp = gwpool.tile([C, 2 * NGEN * C], mybir.dt.bfloat16,
                                      tag="wg")
                    getattr(nc, sqeng).dma_start(wgp[GROWS:C, :], strip_d[j // 2])
                    pair_tiles[j + 1] = wgp
                else:
                    wgp = pair_tiles.pop(j)
                woff = (j % 2) * NGEN * C
                chunks = [(c0, min(c0 + GCH, NGEN)) for c0 in range(0, NGEN, GCH)]
                pgs = [gpsum.tile([C, (c1 - c0) * C], mybir.dt.float32,
                                  tag="pg", name=f"pg{j}_{c0}")
                       for c0, c1 in chunks]
                for z in range(NZ):
                    for (c0, c1), pg in zip(chunks, pgs):
                        nc.tensor.matmul(
                            pg[:, z::NZ],
                            shifts[:, z * C:(z + 1) * C],
                            sj[:, soff + c0 * SEEDY:soff + c1 * SEEDY],
                            start=(z == 0),
                            stop=(z == NZ - 1),
                        )
                for (c0, c1), pg in zip(chunks, pgs):
                    _copy(cpeng, wgp[0:GROWS, woff + c0 * C:woff + c1 * C],
                          pg[0:GROWS, :])
                return (wj, wgp, woff)

            LOOK = look
            def body(_iv=None):
                wgs = {}
                fwgs = {}
                for j in range(FG):
                    fwgs[j] = fullgen_ch(j)
                if NGEN:
                    for j in range(FG, FG + LOOK):
                        wgs[j] = gen_tiles(j)
                for j in range(CPC):
                    if NGEN and j + LOOK < CPC and j >= FG:
                        wgs[j + LOOK] = gen_tiles(j + LOOK)
                    if j < FG:
                        fwg = fwgs.pop(j)
                        wj = wgt = None
                        woff = 0
                    elif NGEN:
                        wj, wgt, woff = wgs.pop(j)
                        fwg = None
                    else:
                        wj = wpool.tile([C, ND * C], mybir.dt.bfloat16)
                        nc.sync.dma_start(wj[:], w_d[j])
                        fwg = None

                    acc = psum.tile([C, NB * B], mybir.dt.float32)
                    if xtiles:
                        xo = j * NB * B
                        xj = xts[xo // XCH][:, xo % XCH:xo % XCH + NB * B]
                    else:
                        xj = xmov[:, j * NB * B:(j + 1) * NB * B]
                    for d in range(NB):
                        ncols = B * (NB - d)
                        if fwg is not None:
                            wsrc = fwg[:, d * C:(d + 1) * C]
                        elif d < ND:
                            wsrc = wj[:, d * C:(d + 1) * C]
                        else:
                            wsrc = wgt[:, woff + (d - ND) * C:
                                       woff + (d - ND + 1) * C]
                        nc.tensor.matmul(
                            acc[:, d * B:],
                            wsrc,
                            xj[:, :ncols],
                            start=(d == 0),
                            stop=(d == NB - 1),
                        )

                    if j % OB == 0:
                        og = opool.tile([C, OB * NB * B], odt, tag="og")
                    _copy(ceng,
                          og[:, (j % OB) * NB * B:(j % OB + 1) * NB * B],
                          acc[:])
                    if j % OB == OB - 1:
                        j0 = j - (OB - 1)
                        if tailsplit and j == CPC - 1:
                            for h in range(OB):
                                getattr(nc, oeng).dma_start(
                                    o_d[:, (j0 + h) * NB * B:
                                        (j0 + h + 1) * NB * B],
                                    og[:, h * NB * B:(h + 1) * NB * B])
                        else:
                            getattr(nc, oeng).dma_start(
                                o_d[:, j0 * NB * B:(j0 + OB) * NB * B],
                                og[:])

            if reps == 1:
                body()
            else:
                with tc.For_i(0, reps, 1) as iv:
                    body(iv)

    nc.compile()
    return nc


def _prep_inputs(P, g, gend0=None, nz=None, fullgen=None):
    if gend0 is None:
        gend0 = KCFG.get("gend0")
    if nz is None:
        nz = KCFG.get("nz", 4)
    if fullgen is None:
        fullgen = KCFG.get("fullgen", 0)
    bf16 = ml_dtypes.bfloat16
    P = np.asarray(P)
    g = np.asarray(g)
    NZ = nz
    SEEDY = C // NZ
    NGEN = NB - gend0 if gend0 is not None else 0
    ND = NB - NGEN
    FG = fullgen if NGEN else 0

    gmod = g.astype(np.float32).copy()
    gmod[0, :] += 1.0

    if NGEN:
        shifts = np.zeros((C, NZ * C), dtype=bf16)
        for z in range(NZ):
            shifts[:, z * C:(z + 1) * C] = np.eye(C, k=-z, dtype=np.float32)

    in_maps = []
    for core in range(N_CORES):
        lo, hi = core * CPC, (core + 1) * CPC
        gpads = np.zeros((CPC, GLEN), dtype=np.float32)
        gpads[:, 127:127 + T] = gmod[:, lo:hi].T
        gpads = gpads.astype(bf16)

        sw = np.lib.stride_tricks.sliding_window_view(gpads, ND * C, axis=1)
        wdense = np.ascontiguousarray(sw[:, :C, :])

        Pc = P[:, :, lo:hi]
        x4 = Pc.reshape(B, NB, C, CPC)
        xmov = np.ascontiguousarray(
            x4[:, :, ::-1, :].transpose(2, 3, 1, 0)
        ).reshape(C, COLS).astype(bf16)

        m = {"xmov": xmov, "wdense": wdense}
        if NGEN:
            seeds = np.empty((CPC, C, NGEN * SEEDY), dtype=bf16)
            strips = np.empty((CPC, NZ - 1, NGEN * C), dtype=bf16)
            pidx = np.arange(C)[:, None]
            yidx = np.arange(SEEDY)[None, :]
            sidx = np.arange(NZ - 1)[:, None]
            cidx = np.arange(C)[None, :]
            for dd in range(NGEN):
                d = ND + dd
                seeds[:, :, dd * SEEDY:(dd + 1) * SEEDY] = \
                    gpads[:, d * C + pidx + NZ * yidx]
                strips[:, :, dd * C:(dd + 1) * C] = \
                    gpads[:, d * C + (C - (NZ - 1) + sidx) + cidx]
            m["wdense"] = np.ascontiguousarray(
                np.concatenate([wdense, seeds], axis=2)[FG:])
            m["strips"] = np.ascontiguousarray(
                strips.reshape(CPC // 2, 2, NZ - 1, NGEN * C)
                .transpose(0, 2, 1, 3).reshape(CPC // 2, NZ - 1, 2 * NGEN * C))
            m["shifts"] = shifts
            if FG:
                fseeds = np.empty((FG, C, NB * SEEDY), dtype=bf16)
                fstrips = np.empty((FG, NZ - 1, NB * C), dtype=bf16)
                for dd in range(NB):
                    fseeds[:, :, dd * SEEDY:(dd + 1) * SEEDY] = \
                        gpads[:FG, dd * C + pidx + NZ * yidx]
                    fstrips[:, :, dd * C:(dd + 1) * C] = \
                        gpads[:FG, dd * C + (C - (NZ - 1) + sidx) + cidx]
                m["fseeds"] = fseeds
                m["fstrips"] = fstrips
        in_maps.append(m)
    return in_maps


def _unshard(results):
    out = np.empty((B, T, NR), np.float32)
    for core in range(N_CORES):
        oc = np.asarray(results[core]["out"], dtype=np.float32)
        oc = oc.reshape(C, CPC, NB, B).transpose(3, 2, 0, 1)
        out[:, :, core * CPC:(core + 1) * CPC] = oc.reshape(B, T, CPC)
    return out


KCFG = dict(OB=2, XSPLIT=4, wbufs=16, obf16=True, warmup=0, gend0=24, nz=4,
            pbufs=4, gpbufs=4, geng="alt", sqeng="gpsimd", look=4, fullgen=0,
            xtiles=1)


def kernel(P, g):
    from concourse.bass_utils import run_bass_kernel_spmd

    if "nc" not in _cache:
        _cache["nc"] = _build_nc(**KCFG)
    nc = _cache["nc"]

    in_maps = _prep_inputs(P, g, gend0=KCFG.get("gend0"),
                           nz=KCFG.get("nz", 4),
                           fullgen=KCFG.get("fullgen", 0))
    res = run_bass_kernel_spmd(nc, in_maps, list(range(N_CORES)))
    return _unshard(res.results)
